# revision 12
# baseline (speedup 1.0000x reference)
"""Trainium2 Bass kernel for the DeepRNN network:

    VanillaRNN(32) -> LSTM(64) -> LSTM(64) -> GRU(64) over T=512 steps,
    then MLP(32,32,12) head and policy = emb @ successor_features.

Strategy
--------
Data-parallel over the batch dim: B=512 sharded as 64 per NeuronCore
(8 cores), weights replicated.

On-device layout is feature-on-partition / batch-on-free:
  state h  ->  [H, B_local] so every matmul is
  psum[M, B] = lhsT(W [K, M]).T @ rhs(h [K, B]).

The recurrent loop is software-pipelined as a 4-deep layer wavefront:
iteration i runs Van(t=i), LSTM1(t=i-1), LSTM2(t=i-2), GRU(t=i-3), which
makes all gate matmuls of an iteration depend only on *previous*
iteration outputs.  That lets all sigmoid gates of one iteration be a
single Scalar-engine activation over one contiguous PSUM region.

Gate packing (free offsets inside the [128, 320] gates PSUM tile):
  IF1 0:64 | IF2 64:128 | OG1 128:192 | OG2 192:256 | ZR 256:320
  partitions: if-banks [f 0:64 ; i 64:128], og-banks [o 0:64 ; g 64:128],
  zr [z 0:64 ; r 64:128].
Biases are folded into the recurrent matmuls via a ones-augmented state
row (lhsT gets an extra bias row, rhs state tiles carry a constant-1
partition); the LSTM forget-gate +1 is folded into that bias row.  The
VanillaRNN bias rides the DVE relu (tensor_scalar add+max).

The big input projection x_t @ van_wi streams the 256 MB observation
tensor from HBM: the host pre-transposes/casts each shard to
[2, 128(d), T*B] bf16 so x_t^T tiles are directly usable as matmul rhs.
"""

import json
import os

import numpy as np
import ml_dtypes

import concourse.bass as bass
import concourse.mybir as mybir
import concourse.tile as tile
from concourse.bass_utils import run_bass_kernel_spmd

# ---------------------------------------------------------------------
# Workaround for a tile<->walrus mismatch in this container: walrus
# rejects instructions carrying more than 2 sync waits ("Too many sync
# wait commands"), but Tile's tail drains aggregate 3+.  Split excess
# waits onto preceding single-wait EventSemaphore instructions on the
# same engine (same program order => same semantics).
_MAXW = 1


def _split_waits(bir_json):
    m = json.loads(bir_json)
    for fn in m.get("functions", []):
        for bb in fn.get("blocks", []) or []:
            insts = bb.get("instructions")
            if not insts:
                continue
            out = []
            for ins in insts:
                si = ins.get("sync_info")
                waits = (si or {}).get("on_wait") or []
                if len(waits) > _MAXW:
                    for k, wt in enumerate(waits[:-_MAXW]):
                        out.append({
                            "debug": ins.get("debug", 0),
                            "engine": ins["engine"],
                            "ins": [],
                            "name": f"{ins['name']}_sw{k}",
                            "opcode": "EventSemaphore",
                            "outs": [],
                            "sync_info": {"on_update": [], "on_wait": [wt]},
                        })
                    si["on_wait"] = waits[-_MAXW:]
                out.append(ins)
            bb["instructions"] = out
    return json.dumps(m).encode()


def _install_wait_split():
    import concourse.bass_utils as bu
    import concourse.bass2jax as b2j

    orig = bu.compile_bir_kernel
    if getattr(orig, "_wait_split_wrapped", False):
        return

    def patched(bir_json, tmpdir, neff_name="file.neff"):
        return orig(_split_waits(bir_json), tmpdir, neff_name=neff_name)

    patched._wait_split_wrapped = True
    bu.compile_bir_kernel = patched
    b2j.compile_bir_kernel = patched


_install_wait_split()


def _install_ntff_hook():
    """The container's antenv package lacks axon_hooks; provide it and
    register the ctypes NTFF profile hook from trn_agent_boot so
    trace=True works (used by test.py for HW timing; harmless
    otherwise)."""
    try:
        import sys
        import types

        try:
            from antenv import axon_hooks  # noqa: F401
            return  # real module exists
        except ImportError:
            pass
        mod = types.ModuleType("antenv.axon_hooks")
        _h = [None]
        mod.set_axon_ntff_profile_hook = lambda h: _h.__setitem__(0, h)
        mod.get_axon_ntff_profile_hook = lambda: _h[0]
        sys.modules["antenv.axon_hooks"] = mod
        import antenv
        antenv.axon_hooks = mod
        from trn_agent_boot.trn_boot import _ntff_profile_via_ctypes
        hook = _ntff_profile_via_ctypes("/opt/axon/libaxon_pjrt.so")
        if hook is not None:
            mod.set_axon_ntff_profile_hook(hook)
    except Exception:
        pass


_install_ntff_hook()

F32 = mybir.dt.float32
BF16 = mybir.dt.bfloat16
AF = mybir.ActivationFunctionType
OP = mybir.AluOpType

B, T, D = 512, 512, 256
H0, H1, H2, H3, NCUM = 32, 64, 64, 64, 12
NCORES = 8
BL = B // NCORES  # 64 batch rows per core

_CACHE = {}


def _build_bass():
    nc = bass.Bass(trn_type="TRN2")
    f32, bf16 = F32, BF16

    # ---- DRAM I/O ----------------------------------------------------
    xdr = nc.dram_tensor("x", [2, 128, T * BL], bf16, kind="ExternalInput")

    wdr = {}
    for name, shape, dt in [
        ("wvi0", [128, H0], bf16), ("wvi1", [128, H0], bf16),
        ("wvh", [H0, H0], f32), ("bv", [H0, 1], f32),
        ("w1x_if", [H0, 128], f32), ("w1h_if", [H1 + 1, 128], f32),
        ("w1x_og", [H0, 128], f32), ("w1h_og", [H1 + 1, 128], f32),
        ("w2x_if", [H1, 128], f32), ("w2h_if", [H2 + 1, 128], f32),
        ("w2x_og", [H1, 128], f32), ("w2h_og", [H2 + 1, 128], f32),
        ("wzr_i", [H2, 128], f32), ("wzr_h", [H3 + 1, 128], f32),
        ("wa_i", [H2, H3], f32), ("wa_h", [H3 + 1, H3], f32),
        ("mw1", [H3, 32], f32), ("mb1", [32, 1], f32),
        ("mw2", [32, 32], f32), ("mb2", [32, 1], f32),
        ("mw3", [32, NCUM], f32), ("mb3", [NCUM, 1], f32),
        ("sv", [NCUM, 1], f32),
    ]:
        wdr[name] = nc.dram_tensor(name, shape, dt, kind="ExternalInput")

    out_dr = nc.dram_tensor("policy", [1, BL], f32, kind="ExternalOutput")

    with tile.TileContext(nc) as tc:
        with (
            tc.tile_pool(name="consts", bufs=1) as consts,
            tc.tile_pool(name="xpool", bufs=1) as xpool,
            tc.tile_pool(name="state", bufs=1) as state,
            tc.tile_pool(name="work", bufs=3) as work,
            tc.tile_pool(name="gpsum", bufs=2, space="PSUM") as gpsum,
            tc.tile_pool(name="spsum", bufs=2, space="PSUM") as spsum,
            tc.tile_pool(name="cpsum", bufs=1, space="PSUM") as cpsum,
        ):
            # ---- load weights ----------------------------------------
            w = {}
            for name, dr in wdr.items():
                wt = consts.tile(list(dr.shape), dr.dtype, name=f"w_{name}")
                nc.sync.dma_start(out=wt, in_=dr[:, :])
                w[name] = wt

            # ---- load x (full residency, chunked DMA) ----------------
            xk0 = xpool.tile([128, T * BL], bf16, name="xk0")
            xk1 = xpool.tile([128, T * BL], bf16, name="xk1")
            NCH = 16
            CW = T * BL // NCH
            for c in range(NCH):
                sl = slice(c * CW, (c + 1) * CW)
                nc.sync.dma_start(out=xk0[:, sl], in_=xdr[0, :, sl])
                nc.sync.dma_start(out=xk1[:, sl], in_=xdr[1, :, sl])

            # ---- persistent state tiles ------------------------------
            h0t = state.tile([H0, BL], f32, name="h0t")
            h12 = state.tile([H1 + 1, 2 * BL], f32, name="h12")  # h1|h2 +ones
            h3t = state.tile([H3 + 1, BL], f32, name="h3t")
            rha = state.tile([H3 + 1, BL], f32, name="rha")
            # LSTM cell states c1|c2 live in PSUM so the f*c + i*g combine
            # obeys the both-SB-inputs-equal-base-partition ISA rule.
            c_ps = cpsum.tile([64, 2 * BL], f32, name="c_ps")

            nc.vector.memset(h0t, 0.0)
            nc.vector.memset(h12[0:H1, :], 0.0)
            nc.vector.memset(h12[H1 : H1 + 1, :], 1.0)
            nc.vector.memset(h3t[0:H3, :], 0.0)
            nc.vector.memset(h3t[H3 : H3 + 1, :], 1.0)
            nc.vector.memset(rha[H3 : H3 + 1, :], 1.0)
            nc.vector.memset(c_ps, 0.0)

            mm = nc.tensor.matmul
            act = nc.scalar.activation

            # ---- wavefront loop --------------------------------------
            for i in range(T + 3):
                van_on = i < T
                l1_on = 0 <= i - 1 < T
                l2_on = 0 <= i - 2 < T
                gru_on = 0 <= i - 3 < T

                if l1_on or l2_on or gru_on:
                    gates = gpsum.tile([128, 320], f32, name="gates")
                    sigs = work.tile([128, 320], f32, name="sigs")

                # --- matmul block (reads previous-iteration state) ----
                if van_on:
                    vps = spsum.tile([H0, BL], f32, name="vps", tag="vps",
                                     bufs=1)
                    xsl = slice(i * BL, (i + 1) * BL)
                    mm(vps, w["wvi0"], xk0[:, xsl], start=True, stop=False)
                    mm(vps, w["wvi1"], xk1[:, xsl], start=False, stop=False)
                    mm(vps, w["wvh"], h0t, start=False, stop=True)
                if l1_on:
                    mm(gates[:, 0:64], w["w1x_if"], h0t, start=True, stop=False)
                    mm(gates[:, 0:64], w["w1h_if"], h12[:, 0:BL],
                       start=False, stop=True)
                    mm(gates[:, 128:192], w["w1x_og"], h0t,
                       start=True, stop=False)
                    mm(gates[:, 128:192], w["w1h_og"], h12[:, 0:BL],
                       start=False, stop=True)
                if l2_on:
                    mm(gates[:, 64:128], w["w2x_if"], h12[0:H1, 0:BL],
                       start=True, stop=False)
                    mm(gates[:, 64:128], w["w2h_if"], h12[:, BL : 2 * BL],
                       start=False, stop=True)
                    mm(gates[:, 192:256], w["w2x_og"], h12[0:H1, 0:BL],
                       start=True, stop=False)
                    mm(gates[:, 192:256], w["w2h_og"], h12[:, BL : 2 * BL],
                       start=False, stop=True)
                if gru_on:
                    mm(gates[:, 256:320], w["wzr_i"], h12[0:H1, BL : 2 * BL],
                       start=True, stop=False)
                    mm(gates[:, 256:320], w["wzr_h"], h3t,
                       start=False, stop=True)
                    aps = spsum.tile([H3, BL], f32, name="aps", tag="aps",
                                     bufs=1)
                    mm(aps, w["wa_i"], h12[0:H1, BL : 2 * BL],
                       start=True, stop=False)

                # --- van relu (updates h0); bias via ACT per-partition
                if van_on:
                    act(h0t, vps, AF.Relu, bias=w["bv"])

                # --- one sigmoid over merged active gate intervals ----
                ivs = []
                if l1_on:
                    ivs += [(0, 64), (128, 192)]
                if l2_on:
                    ivs += [(64, 128), (192, 256)]
                if gru_on:
                    ivs += [(256, 320)]
                ivs.sort()
                merged = []
                for s, e in ivs:
                    if merged and merged[-1][1] == s:
                        merged[-1][1] = e
                    else:
                        merged.append([s, e])
                for s, e in merged:
                    act(sigs[:, s:e], gates[:, s:e], AF.Sigmoid)

                # --- tanh(g) into g128[64:128] (base 64 = gate base) --
                if l1_on or l2_on:
                    glo = 0 if l1_on else BL
                    ghi = 2 * BL if l2_on else BL
                    g128 = work.tile([128, 2 * BL], f32, name="g128")
                    act(g128[64:128, glo:ghi],
                        gates[64:128, 128 + glo : 128 + ghi], AF.Tanh)

                # --- LSTM elementwise (both layers in one op) ---------
                if l1_on or l2_on:
                    lo, hi = glo, ghi
                    fc_t = work.tile([64, 2 * BL], f32, name="fc_t")
                    ig_ps = spsum.tile([64, 2 * BL], f32, name="ig_ps",
                                       tag="ig_ps", bufs=1)
                    tcn = work.tile([64, 2 * BL], f32, name="tcn")
                    # f~ * c   (SB0 x PSUM0 -> SB0)
                    nc.vector.tensor_mul(
                        fc_t[:, lo:hi], sigs[0:64, lo:hi], c_ps[:, lo:hi])
                    # i~ * g~  (SB64 x SB64 -> PSUM0)
                    nc.vector.tensor_mul(
                        ig_ps[:, lo:hi], sigs[64:128, lo:hi],
                        g128[64:128, lo:hi])
                    # c = f~c + i~g~  (SB0 + PSUM0 -> PSUM0)
                    nc.vector.tensor_add(
                        c_ps[:, lo:hi], fc_t[:, lo:hi], ig_ps[:, lo:hi])
                    act(tcn[:, lo:hi], c_ps[:, lo:hi], AF.Tanh)
                    nc.vector.tensor_mul(
                        h12[0:64, lo:hi], sigs[0:64, 128 + lo : 128 + hi],
                        tcn[:, lo:hi])

                # --- GRU tail (zr packed as [r; z]) -------------------
                if gru_on:
                    a_s = work.tile([H3, BL], f32, name="a_s")
                    d_ps = spsum.tile([H3, BL], f32, name="d_ps",
                                      tag="d_ps", bufs=1)
                    zd = work.tile([H3, BL], f32, name="zd")
                    nc.vector.tensor_mul(
                        rha[0:H3, :], sigs[0:64, 256:320], h3t[0:H3, :])
                    mm(aps, w["wa_h"], rha, start=False, stop=True)
                    act(a_s, aps, AF.Tanh)
                    nc.vector.tensor_sub(d_ps, a_s, h3t[0:H3, :])
                    nc.vector.tensor_mul(zd, sigs[64:128, 256:320], d_ps)
                    nc.vector.tensor_add(h3t[0:H3, :], h3t[0:H3, :], zd)

            # ---- MLP head + policy -----------------------------------
            y1p = spsum.tile([32, BL], f32, name="y1p", tag="tail", bufs=1)
            mm(y1p, w["mw1"], h3t[0:H3, :], start=True, stop=True)
            y1s = work.tile([32, BL], f32, name="y1s")
            nc.vector.tensor_scalar(
                out=y1s, in0=y1p, scalar1=w["mb1"], scalar2=0.0,
                op0=OP.add, op1=OP.max)

            y2p = spsum.tile([32, BL], f32, name="y2p", tag="tail", bufs=1)
            mm(y2p, w["mw2"], y1s, start=True, stop=True)
            y2s = work.tile([32, BL], f32, name="y2s")
            nc.vector.tensor_scalar(
                out=y2s, in0=y2p, scalar1=w["mb2"], scalar2=0.0,
                op0=OP.add, op1=OP.max)

            y3p = spsum.tile([NCUM, BL], f32, name="y3p", tag="tail", bufs=1)
            mm(y3p, w["mw3"], y2s, start=True, stop=True)
            y3s = work.tile([NCUM, BL], f32, name="y3s")
            nc.vector.tensor_scalar_add(y3s, y3p, w["mb3"])

            pp = spsum.tile([1, BL], f32, name="pp", tag="tail", bufs=1)
            mm(pp, w["sv"], y3s, start=True, stop=True)
            pols = work.tile([1, BL], f32, name="pols")
            nc.vector.tensor_copy(out=pols, in_=pp)
            nc.sync.dma_start(out=out_dr[:, :], in_=pols)

    return nc


def _pack_weights(inp):
    f32 = np.float32

    def arr(name):
        return np.ascontiguousarray(np.asarray(inp[name], dtype=f32))

    out = {}
    wvi = arr("van_wi").astype(ml_dtypes.bfloat16)
    out["wvi0"], out["wvi1"] = (np.ascontiguousarray(wvi[:128]),
                                np.ascontiguousarray(wvi[128:]))
    out["wvh"] = arr("van_wh")
    out["bv"] = (arr("van_bi") + arr("van_bh")).reshape(H0, 1)

    def lstm_pack(wname, bname, xdim, pfx):
        wf = arr(wname)
        bf = arr(bname)
        i, g, f, o = np.split(wf, 4, axis=1)
        bi, bg, bfo, bo = np.split(bf, 4)
        w_if = np.concatenate([f, i], axis=1)
        w_og = np.concatenate([o, g], axis=1)
        b_if = np.concatenate([bfo + 1.0, bi])
        b_og = np.concatenate([bo, bg])
        out[f"{pfx}x_if"] = np.ascontiguousarray(w_if[:xdim])
        out[f"{pfx}h_if"] = np.ascontiguousarray(
            np.concatenate([w_if[xdim:], b_if[None, :]], axis=0))
        out[f"{pfx}x_og"] = np.ascontiguousarray(w_og[:xdim])
        out[f"{pfx}h_og"] = np.ascontiguousarray(
            np.concatenate([w_og[xdim:], b_og[None, :]], axis=0))

    lstm_pack("lstm1_w", "lstm1_b", H0, "w1")
    lstm_pack("lstm2_w", "lstm2_b", H1, "w2")

    gwi, gwh, gb = arr("gru_wi"), arr("gru_wh"), arr("gru_b")
    # zr block repacked as [r; z] so r lands on partitions 0:64 (rh mult
    # against h3 needs equal SB base partitions)
    zperm = np.concatenate([np.arange(H3, 2 * H3), np.arange(0, H3)])
    out["wzr_i"] = np.ascontiguousarray(gwi[:, zperm])
    out["wzr_h"] = np.ascontiguousarray(
        np.concatenate([gwh[:, zperm], gb[None, zperm]], axis=0))
    out["wa_i"] = np.ascontiguousarray(gwi[:, 2 * H3 :])
    out["wa_h"] = np.ascontiguousarray(
        np.concatenate([gwh[:, 2 * H3 :], gb[None, 2 * H3 :]], axis=0))

    out["mw1"] = arr("mlp_w1")
    out["mb1"] = arr("mlp_b1").reshape(32, 1)
    out["mw2"] = arr("mlp_w2")
    out["mb2"] = arr("mlp_b2").reshape(32, 1)
    out["mw3"] = arr("mlp_w3")
    out["mb3"] = arr("mlp_b3").reshape(NCUM, 1)
    out["sv"] = arr("successor_features").reshape(NCUM, 1)
    return out


def kernel(**inputs):
    assert int(inputs["action_index"]) == 0

    X = np.asarray(inputs["pixels_observation"], dtype=np.float32)
    assert X.shape == (B, T, D), X.shape

    wmaps = _pack_weights(inputs)

    in_maps = []
    for c in range(NCORES):
        xc = X[c * BL : (c + 1) * BL]                # [BL, T, D]
        xt = xc.transpose(2, 1, 0)                   # [D, T, BL]
        xt = xt.reshape(2, 128, T * BL).astype(ml_dtypes.bfloat16)
        m = dict(wmaps)
        m["x"] = np.ascontiguousarray(xt)
        in_maps.append(m)

    if "nc" not in _CACHE:
        _CACHE["nc"] = _build_bass()
    nc = _CACHE["nc"]

    trace = bool(int(os.environ.get("KERNEL_TRACE", "0")))
    res = run_bass_kernel_spmd(
        nc, in_maps, core_ids=list(range(NCORES)), trace=trace)
    if trace and res.exec_time_ns is not None:
        print(f"HW exec time: {res.exec_time_ns} ns")
        _CACHE["exec_time_ns"] = res.exec_time_ns

    policy = np.zeros((B, 1), np.float32)
    for c in range(NCORES):
        policy[c * BL : (c + 1) * BL, 0] = res.results[c]["policy"][0]

    succ = np.ascontiguousarray(
        np.asarray(inputs["successor_features"], dtype=np.float32))
    pref = np.ones((NCUM, 1), np.float32)
    return succ, pref, policy


# revision 13
# speedup vs baseline: 1.7150x; 1.7150x over previous
"""Trainium2 Bass kernel for the DeepRNN network:

    VanillaRNN(32) -> LSTM(64) -> LSTM(64) -> GRU(64) over T=512 steps,
    then MLP(32,32,12) head and policy = emb @ successor_features.

Strategy
--------
Data-parallel over the batch dim: B=512 sharded as 64 per NeuronCore
(8 cores), weights replicated.

On-device layout is feature-on-partition / batch-on-free:
  state h  ->  [H, B_local] so every matmul is
  psum[M, B] = lhsT(W [K, M]).T @ rhs(h [K, B]).

The recurrent loop is software-pipelined as a 4-deep layer wavefront:
iteration i runs Van(t=i), LSTM1(t=i-1), LSTM2(t=i-2), GRU(t=i-3), which
makes all gate matmuls of an iteration depend only on *previous*
iteration outputs.  That lets all sigmoid gates of one iteration be a
single Scalar-engine activation over one contiguous PSUM region.

Gate packing (free offsets inside the [128, 320] gates PSUM tile):
  IF1 0:64 | IF2 64:128 | OG1 128:192 | OG2 192:256 | ZR 256:320
  partitions: if-banks [f 0:64 ; i 64:128], og-banks [o 0:64 ; g 64:128],
  zr [z 0:64 ; r 64:128].
Biases are folded into the recurrent matmuls via a ones-augmented state
row (lhsT gets an extra bias row, rhs state tiles carry a constant-1
partition); the LSTM forget-gate +1 is folded into that bias row.  The
VanillaRNN bias rides the DVE relu (tensor_scalar add+max).

The big input projection x_t @ van_wi streams the 256 MB observation
tensor from HBM: the host pre-transposes/casts each shard to
[2, 128(d), T*B] bf16 so x_t^T tiles are directly usable as matmul rhs.
"""

import json
import os

import numpy as np
import ml_dtypes

import concourse.bass as bass
import concourse.mybir as mybir
import concourse.tile as tile
from concourse.bass_utils import run_bass_kernel_spmd

# ---------------------------------------------------------------------
# Workaround for a tile<->walrus mismatch in this container: walrus
# rejects instructions carrying more than 2 sync waits ("Too many sync
# wait commands"), but Tile's tail drains aggregate 3+.  Split excess
# waits onto preceding single-wait EventSemaphore instructions on the
# same engine (same program order => same semantics).
_MAXW = 1


def _split_waits(bir_json):
    m = json.loads(bir_json)
    for fn in m.get("functions", []):
        for bb in fn.get("blocks", []) or []:
            insts = bb.get("instructions")
            if not insts:
                continue
            out = []
            for ins in insts:
                si = ins.get("sync_info")
                waits = (si or {}).get("on_wait") or []
                if len(waits) > _MAXW:
                    for k, wt in enumerate(waits[:-_MAXW]):
                        out.append({
                            "debug": ins.get("debug", 0),
                            "engine": ins["engine"],
                            "ins": [],
                            "name": f"{ins['name']}_sw{k}",
                            "opcode": "EventSemaphore",
                            "outs": [],
                            "sync_info": {"on_update": [], "on_wait": [wt]},
                        })
                    si["on_wait"] = waits[-_MAXW:]
                out.append(ins)
            bb["instructions"] = out
    return json.dumps(m).encode()


def _install_wait_split():
    import concourse.bass_utils as bu
    import concourse.bass2jax as b2j

    orig = bu.compile_bir_kernel
    if getattr(orig, "_wait_split_wrapped", False):
        return

    def patched(bir_json, tmpdir, neff_name="file.neff"):
        return orig(_split_waits(bir_json), tmpdir, neff_name=neff_name)

    patched._wait_split_wrapped = True
    bu.compile_bir_kernel = patched
    b2j.compile_bir_kernel = patched


_install_wait_split()


def _install_ntff_hook():
    """The container's antenv package lacks axon_hooks; provide it and
    register the ctypes NTFF profile hook from trn_agent_boot so
    trace=True works (used by test.py for HW timing; harmless
    otherwise)."""
    try:
        import sys
        import types

        try:
            from antenv import axon_hooks  # noqa: F401
            return  # real module exists
        except ImportError:
            pass
        mod = types.ModuleType("antenv.axon_hooks")
        _h = [None]
        mod.set_axon_ntff_profile_hook = lambda h: _h.__setitem__(0, h)
        mod.get_axon_ntff_profile_hook = lambda: _h[0]
        sys.modules["antenv.axon_hooks"] = mod
        import antenv
        antenv.axon_hooks = mod
        from trn_agent_boot.trn_boot import _ntff_profile_via_ctypes
        hook = _ntff_profile_via_ctypes("/opt/axon/libaxon_pjrt.so")
        if hook is not None:
            mod.set_axon_ntff_profile_hook(hook)
    except Exception:
        pass


_install_ntff_hook()

F32 = mybir.dt.float32
BF16 = mybir.dt.bfloat16
AF = mybir.ActivationFunctionType
OP = mybir.AluOpType

B, T, D = 512, 512, 256
H0, H1, H2, H3, NCUM = 32, 64, 64, 64, 12
NCORES = 8
BL = B // NCORES  # 64 batch rows per core

_CACHE = {}


def _build_bass():
    nc = bass.Bass(trn_type="TRN2")
    f32, bf16 = F32, BF16

    # ---- DRAM I/O ----------------------------------------------------
    xdr = nc.dram_tensor("x", [2, 128, T * BL], bf16, kind="ExternalInput")

    wdr = {}
    for name, shape, dt in [
        ("wvi0", [128, H0], bf16), ("wvi1", [128, H0], bf16),
        ("wvh", [H0, H0], bf16), ("bv", [H0, 1], f32),
        ("w1x_if", [H0, 128], bf16), ("w1h_if", [H1 + 1, 128], bf16),
        ("w1x_og", [H0, 128], bf16), ("w1h_og", [H1 + 1, 128], bf16),
        ("w2x_if", [H1, 128], bf16), ("w2h_if", [H2 + 1, 128], bf16),
        ("w2x_og", [H1, 128], bf16), ("w2h_og", [H2 + 1, 128], bf16),
        ("wzr_i", [H2, 128], bf16), ("wzr_h", [H3 + 1, 128], bf16),
        ("wa_i", [H2, H3], bf16), ("wa_h", [H3 + 1, H3], bf16),
        ("mw1", [H3, 32], f32), ("mb1", [32, 1], f32),
        ("mw2", [32, 32], f32), ("mb2", [32, 1], f32),
        ("mw3", [32, NCUM], f32), ("mb3", [NCUM, 1], f32),
        ("sv", [NCUM, 1], f32),
    ]:
        wdr[name] = nc.dram_tensor(name, shape, dt, kind="ExternalInput")

    out_dr = nc.dram_tensor("policy", [1, BL], f32, kind="ExternalOutput")

    with tile.TileContext(nc) as tc:
        with (
            tc.tile_pool(name="consts", bufs=1) as consts,
            tc.tile_pool(name="xpool", bufs=1) as xpool,
            tc.tile_pool(name="state", bufs=1) as state,
            tc.tile_pool(name="work", bufs=3) as work,
            tc.tile_pool(name="gpsum", bufs=2, space="PSUM") as gpsum,
            tc.tile_pool(name="spsum", bufs=2, space="PSUM") as spsum,
            tc.tile_pool(name="cpsum", bufs=1, space="PSUM") as cpsum,
        ):
            # ---- load weights ----------------------------------------
            w = {}
            for name, dr in wdr.items():
                wt = consts.tile(list(dr.shape), dr.dtype, name=f"w_{name}")
                nc.sync.dma_start(out=wt, in_=dr[:, :])
                w[name] = wt

            # ---- load x (full residency, chunked DMA) ----------------
            xk0 = xpool.tile([128, T * BL], bf16, name="xk0")
            xk1 = xpool.tile([128, T * BL], bf16, name="xk1")
            NCH = 16
            CW = T * BL // NCH
            for c in range(NCH):
                sl = slice(c * CW, (c + 1) * CW)
                nc.sync.dma_start(out=xk0[:, sl], in_=xdr[0, :, sl])
                nc.sync.dma_start(out=xk1[:, sl], in_=xdr[1, :, sl])

            # ---- persistent state tiles ------------------------------
            h0t = state.tile([H0, BL], bf16, name="h0t")
            h12 = state.tile([H1 + 1, 2 * BL], bf16, name="h12")  # h1|h2 +1s
            h3t = state.tile([H3 + 1, BL], f32, name="h3t")
            h3b = state.tile([H3 + 1, BL], bf16, name="h3b")  # matmul shadow
            rha = state.tile([H3 + 1, BL], bf16, name="rha")
            # LSTM cell states c1|c2 live in PSUM so the f*c + i*g combine
            # obeys the both-SB-inputs-equal-base-partition ISA rule.
            c_ps = cpsum.tile([64, 2 * BL], f32, name="c_ps")

            nc.vector.memset(h0t, 0.0)
            nc.vector.memset(h12[0:H1, :], 0.0)
            nc.vector.memset(h12[H1 : H1 + 1, :], 1.0)
            nc.vector.memset(h3t[0:H3, :], 0.0)
            nc.vector.memset(h3t[H3 : H3 + 1, :], 1.0)
            nc.vector.memset(h3b[0:H3, :], 0.0)
            nc.vector.memset(h3b[H3 : H3 + 1, :], 1.0)
            nc.vector.memset(rha[H3 : H3 + 1, :], 1.0)
            nc.vector.memset(c_ps, 0.0)

            mm = nc.tensor.matmul
            act = nc.scalar.activation

            # ---- wavefront loop --------------------------------------
            for i in range(T + 3):
                van_on = i < T
                l1_on = 0 <= i - 1 < T
                l2_on = 0 <= i - 2 < T
                gru_on = 0 <= i - 3 < T

                if l1_on or l2_on or gru_on:
                    gates = gpsum.tile([128, 320], f32, name="gates")
                    sigs = work.tile([128, 320], f32, name="sigs")

                # --- matmul block (reads previous-iteration state) ----
                if van_on:
                    vps = spsum.tile([H0, BL], f32, name="vps", tag="vps",
                                     bufs=1)
                    xsl = slice(i * BL, (i + 1) * BL)
                    mm(vps, w["wvi0"], xk0[:, xsl], start=True, stop=False)
                    mm(vps, w["wvi1"], xk1[:, xsl], start=False, stop=False)
                    mm(vps, w["wvh"], h0t, start=False, stop=True)
                if l1_on:
                    mm(gates[:, 0:64], w["w1x_if"], h0t, start=True, stop=False)
                    mm(gates[:, 0:64], w["w1h_if"], h12[:, 0:BL],
                       start=False, stop=True)
                    mm(gates[:, 128:192], w["w1x_og"], h0t,
                       start=True, stop=False)
                    mm(gates[:, 128:192], w["w1h_og"], h12[:, 0:BL],
                       start=False, stop=True)
                if l2_on:
                    mm(gates[:, 64:128], w["w2x_if"], h12[0:H1, 0:BL],
                       start=True, stop=False)
                    mm(gates[:, 64:128], w["w2h_if"], h12[:, BL : 2 * BL],
                       start=False, stop=True)
                    mm(gates[:, 192:256], w["w2x_og"], h12[0:H1, 0:BL],
                       start=True, stop=False)
                    mm(gates[:, 192:256], w["w2h_og"], h12[:, BL : 2 * BL],
                       start=False, stop=True)
                if gru_on:
                    mm(gates[:, 256:320], w["wzr_i"], h12[0:H1, BL : 2 * BL],
                       start=True, stop=False)
                    mm(gates[:, 256:320], w["wzr_h"], h3b,
                       start=False, stop=True)
                    aps = spsum.tile([H3, BL], f32, name="aps", tag="aps",
                                     bufs=1)
                    mm(aps, w["wa_i"], h12[0:H1, BL : 2 * BL],
                       start=True, stop=False)

                # --- van relu (updates h0); bias via ACT per-partition
                if van_on:
                    act(h0t, vps, AF.Relu, bias=w["bv"])

                # --- one sigmoid over merged active gate intervals ----
                ivs = []
                if l1_on:
                    ivs += [(0, 64), (128, 192)]
                if l2_on:
                    ivs += [(64, 128), (192, 256)]
                if gru_on:
                    ivs += [(256, 320)]
                ivs.sort()
                merged = []
                for s, e in ivs:
                    if merged and merged[-1][1] == s:
                        merged[-1][1] = e
                    else:
                        merged.append([s, e])
                for s, e in merged:
                    act(sigs[:, s:e], gates[:, s:e], AF.Sigmoid)

                # --- tanh(g) into g128[64:128] (base 64 = gate base) --
                if l1_on or l2_on:
                    glo = 0 if l1_on else BL
                    ghi = 2 * BL if l2_on else BL
                    g128 = work.tile([128, 2 * BL], f32, name="g128")
                    act(g128[64:128, glo:ghi],
                        gates[64:128, 128 + glo : 128 + ghi], AF.Tanh)

                # --- LSTM elementwise (both layers in one op) ---------
                if l1_on or l2_on:
                    lo, hi = glo, ghi
                    fc_t = work.tile([64, 2 * BL], f32, name="fc_t")
                    ig_ps = spsum.tile([64, 2 * BL], f32, name="ig_ps",
                                       tag="ig_ps", bufs=1)
                    tcn = work.tile([64, 2 * BL], f32, name="tcn")
                    # f~ * c   (SB0 x PSUM0 -> SB0)
                    nc.vector.tensor_mul(
                        fc_t[:, lo:hi], sigs[0:64, lo:hi], c_ps[:, lo:hi])
                    # i~ * g~  (SB64 x SB64 -> PSUM0)
                    nc.vector.tensor_mul(
                        ig_ps[:, lo:hi], sigs[64:128, lo:hi],
                        g128[64:128, lo:hi])
                    # c = f~c + i~g~  (SB0 + PSUM0 -> PSUM0)
                    nc.vector.tensor_add(
                        c_ps[:, lo:hi], fc_t[:, lo:hi], ig_ps[:, lo:hi])
                    act(tcn[:, lo:hi], c_ps[:, lo:hi], AF.Tanh)
                    nc.vector.tensor_mul(
                        h12[0:64, lo:hi], sigs[0:64, 128 + lo : 128 + hi],
                        tcn[:, lo:hi])

                # --- GRU tail (zr packed as [r; z]) -------------------
                if gru_on:
                    a_s = work.tile([H3, BL], f32, name="a_s")
                    d_ps = spsum.tile([H3, BL], f32, name="d_ps",
                                      tag="d_ps", bufs=1)
                    zd = work.tile([H3, BL], f32, name="zd")
                    nc.vector.tensor_mul(
                        rha[0:H3, :], sigs[0:64, 256:320], h3t[0:H3, :])
                    mm(aps, w["wa_h"], rha, start=False, stop=True)
                    act(a_s, aps, AF.Tanh)
                    nc.vector.tensor_sub(d_ps, a_s, h3t[0:H3, :])
                    nc.vector.tensor_mul(zd, sigs[64:128, 256:320], d_ps)
                    nc.vector.tensor_add(h3t[0:H3, :], h3t[0:H3, :], zd)
                    nc.vector.tensor_copy(out=h3b[0:H3, :], in_=h3t[0:H3, :])

            # ---- MLP head + policy -----------------------------------
            y1p = spsum.tile([32, BL], f32, name="y1p", tag="tail", bufs=1)
            mm(y1p, w["mw1"], h3t[0:H3, :], start=True, stop=True)
            y1s = work.tile([32, BL], f32, name="y1s")
            nc.vector.tensor_scalar(
                out=y1s, in0=y1p, scalar1=w["mb1"], scalar2=0.0,
                op0=OP.add, op1=OP.max)

            y2p = spsum.tile([32, BL], f32, name="y2p", tag="tail", bufs=1)
            mm(y2p, w["mw2"], y1s, start=True, stop=True)
            y2s = work.tile([32, BL], f32, name="y2s")
            nc.vector.tensor_scalar(
                out=y2s, in0=y2p, scalar1=w["mb2"], scalar2=0.0,
                op0=OP.add, op1=OP.max)

            y3p = spsum.tile([NCUM, BL], f32, name="y3p", tag="tail", bufs=1)
            mm(y3p, w["mw3"], y2s, start=True, stop=True)
            y3s = work.tile([NCUM, BL], f32, name="y3s")
            nc.vector.tensor_scalar_add(y3s, y3p, w["mb3"])

            pp = spsum.tile([1, BL], f32, name="pp", tag="tail", bufs=1)
            mm(pp, w["sv"], y3s, start=True, stop=True)
            pols = work.tile([1, BL], f32, name="pols")
            nc.vector.tensor_copy(out=pols, in_=pp)
            nc.sync.dma_start(out=out_dr[:, :], in_=pols)

    return nc


def _pack_weights(inp):
    f32 = np.float32

    def arr(name):
        return np.ascontiguousarray(np.asarray(inp[name], dtype=f32))

    out = {}
    wvi = arr("van_wi").astype(ml_dtypes.bfloat16)
    out["wvi0"], out["wvi1"] = (np.ascontiguousarray(wvi[:128]),
                                np.ascontiguousarray(wvi[128:]))
    out["wvh"] = arr("van_wh").astype(ml_dtypes.bfloat16)
    out["bv"] = (arr("van_bi") + arr("van_bh")).reshape(H0, 1)

    def lstm_pack(wname, bname, xdim, pfx):
        wf = arr(wname)
        bf = arr(bname)
        i, g, f, o = np.split(wf, 4, axis=1)
        bi, bg, bfo, bo = np.split(bf, 4)
        w_if = np.concatenate([f, i], axis=1)
        w_og = np.concatenate([o, g], axis=1)
        b_if = np.concatenate([bfo + 1.0, bi])
        b_og = np.concatenate([bo, bg])
        bf = ml_dtypes.bfloat16
        out[f"{pfx}x_if"] = np.ascontiguousarray(w_if[:xdim].astype(bf))
        out[f"{pfx}h_if"] = np.ascontiguousarray(
            np.concatenate([w_if[xdim:], b_if[None, :]], axis=0).astype(bf))
        out[f"{pfx}x_og"] = np.ascontiguousarray(w_og[:xdim].astype(bf))
        out[f"{pfx}h_og"] = np.ascontiguousarray(
            np.concatenate([w_og[xdim:], b_og[None, :]], axis=0).astype(bf))

    lstm_pack("lstm1_w", "lstm1_b", H0, "w1")
    lstm_pack("lstm2_w", "lstm2_b", H1, "w2")

    gwi, gwh, gb = arr("gru_wi"), arr("gru_wh"), arr("gru_b")
    # zr block repacked as [r; z] so r lands on partitions 0:64 (rh mult
    # against h3 needs equal SB base partitions)
    zperm = np.concatenate([np.arange(H3, 2 * H3), np.arange(0, H3)])
    bf = ml_dtypes.bfloat16
    out["wzr_i"] = np.ascontiguousarray(gwi[:, zperm].astype(bf))
    out["wzr_h"] = np.ascontiguousarray(
        np.concatenate([gwh[:, zperm], gb[None, zperm]], axis=0).astype(bf))
    out["wa_i"] = np.ascontiguousarray(gwi[:, 2 * H3 :].astype(bf))
    out["wa_h"] = np.ascontiguousarray(
        np.concatenate([gwh[:, 2 * H3 :], gb[None, 2 * H3 :]],
                       axis=0).astype(bf))

    out["mw1"] = arr("mlp_w1")
    out["mb1"] = arr("mlp_b1").reshape(32, 1)
    out["mw2"] = arr("mlp_w2")
    out["mb2"] = arr("mlp_b2").reshape(32, 1)
    out["mw3"] = arr("mlp_w3")
    out["mb3"] = arr("mlp_b3").reshape(NCUM, 1)
    out["sv"] = arr("successor_features").reshape(NCUM, 1)
    return out


def kernel(**inputs):
    assert int(inputs["action_index"]) == 0

    X = np.asarray(inputs["pixels_observation"], dtype=np.float32)
    assert X.shape == (B, T, D), X.shape

    wmaps = _pack_weights(inputs)

    in_maps = []
    for c in range(NCORES):
        xc = X[c * BL : (c + 1) * BL]                # [BL, T, D]
        xt = xc.transpose(2, 1, 0)                   # [D, T, BL]
        xt = xt.reshape(2, 128, T * BL).astype(ml_dtypes.bfloat16)
        m = dict(wmaps)
        m["x"] = np.ascontiguousarray(xt)
        in_maps.append(m)

    if "nc" not in _CACHE:
        _CACHE["nc"] = _build_bass()
    nc = _CACHE["nc"]

    trace = bool(int(os.environ.get("KERNEL_TRACE", "0")))
    res = run_bass_kernel_spmd(
        nc, in_maps, core_ids=list(range(NCORES)), trace=trace)
    if trace and res.exec_time_ns is not None:
        print(f"HW exec time: {res.exec_time_ns} ns")
        _CACHE["exec_time_ns"] = res.exec_time_ns

    policy = np.zeros((B, 1), np.float32)
    for c in range(NCORES):
        policy[c * BL : (c + 1) * BL, 0] = res.results[c]["policy"][0]

    succ = np.ascontiguousarray(
        np.asarray(inputs["successor_features"], dtype=np.float32))
    pref = np.ones((NCUM, 1), np.float32)
    return succ, pref, policy
